# revision 19
# baseline (speedup 1.0000x reference)
"""MoE expert-group kernel for Trainium2 (8 NeuronCores).

Problem: T=2048 tokens, E=8 experts, D=1024, I=2048.
  out[t] = silu(x[t] @ w_gate[e]) * (x[t] @ w_up[e]) @ w_down[e],  e = expert_indices[t]

Strategy: expert parallelism. Host-side (numpy) routing gathers tokens by
expert (this is the "all-to-all"); core e runs expert e's dense
gate/up/silu/down pipeline; host scatters rows back.

On-chip formulation is fully transposed so no transposes are ever needed:
  gateT = Wg^T @ X^T        (stationary = 128x128 Wg block, moving = xT [128, C])
  hidT  = silu(gateT)*upT   (ACT Silu + one DVE mul, written bf16)
  outT  = Wd^T @ hidT       (stationary = 128x128 Wd block, moving = hT [128, C])

Numerics: weights are shipped as int8 (per-expert linear quantization,
scale = absmax/127) and cast to bf16 *inside* the gpsimd DMA engines, so
HBM weight traffic halves while the PE still runs plain bf16 matmuls on
exact integer values. x stays bf16. The gate dequant scale s_g is folded
into the Silu's ACT pre-scale (it sits inside the nonlinearity); the
remaining 1/(s_u*s_d) factor is purely multiplicative, so the output is
written back bf16 UNscaled and the host applies s_u*s_d during the
fp32 upcast. Measured rel err ~8e-3 vs the 2e-2 budget.

Timeline engineering (what the traces showed matters on TRN2):
- The PE clock sits at 1.2GHz until ~3.2us of CONTINUOUS busy, then
  jumps to full rate (HAM 8/8); ANY idle gap drops it back and costs
  ~3.4us of half-rate work. A junk-matmul burst (zeros from a memset
  tile into a scratch PSUM slot) starts right at body entry and is
  sized to overshoot the first real matmul's data arrival, so the
  handoff is seamless and all real matmuls run at full clock.
- DMA queues have ~17ns/row descriptor overhead, so x tiles ([128, C]
  with 0.5-1KB rows) are row-bound, not byte-bound: one x chunk per
  HWDGE ring (sync: d0d1, scalar: d2d3), and d4..d7 ride the SWDGE
  casting ring (fast per-packet) right after the first gate strip.
  Each dma_start also costs ~0.7us of ISSUE time on its engine queue,
  so the doorbell order is part of the schedule.
- SWDGE stream order == consumption order with slack maximized:
  wg0, x4567, wu0, wg1, wu1, then 2-slice pairs, then wd d-major.
- The measured window ends only after a fixed ~8us walrus postamble
  (all-engine barrier + full semaphore-file reset + barrier), so the
  controllable tail is the output drain: copies run bf16 on DVE,
  output strips alternate between the two HWDGE rings, and the final
  d-strip is split by PARTITIONS across both rings (column splits
  don't help: DMA cost is per-row).
"""

import sys

import numpy as np

try:
    import concourse  # noqa: F401
except ImportError:  # grading env fallback
    sys.path.insert(0, "/opt/trn_rl_repo")

import ml_dtypes

T, E, D, I = 2048, 8, 1024, 2048
ND = D // 128  # 8 contraction tiles for gate/up
NI = I // 128  # 16 contraction tiles for down
N_JUNK = 56  # PE warm-up burst length (64-col junk matmuls, ~53ns each)

_PROGRAM_CACHE = {}


def _build_program(C):
    """Build + compile the per-core Bass program for token capacity C."""
    import concourse.bass as bass  # noqa: F401
    import concourse.mybir as mybir
    import concourse.tile as tile
    from concourse import bacc

    BF = mybir.dt.bfloat16
    I8 = mybir.dt.int8
    F32 = mybir.dt.float32

    nc = bacc.Bacc(
        "TRN2",
        target_bir_lowering=False,
        debug=False,
        num_devices=E,
        enable_partition_id=False,
    )
    # x ships as 4 tensors: x01/x23/x45/x67 = [128, 2*C] (d-pairs, slot
    # d'*C+c -> x[tok c, (2j+d')*128+p]); x01/x23 on the HWDGE rings,
    # x45/x67 in the SWDGE stream
    x01_d = nc.dram_tensor("x01", [128, 2 * C], BF, kind="ExternalInput").ap()
    x23_d = nc.dram_tensor("x23", [128, 2 * C], BF, kind="ExternalInput").ap()
    x45_d = nc.dram_tensor("x45", [128, 2 * C], BF, kind="ExternalInput").ap()
    x67_d = nc.dram_tensor("x67", [128, 2 * C], BF, kind="ExternalInput").ap()
    # wg/wu packed: [128, NI*D] int8, free slot i*D + d*128 + q  <-
    #   round(w[d*128+p, i*128+q]/s) for the [D, I] projections
    wg_d = nc.dram_tensor("wg", [128, NI * D], I8, kind="ExternalInput").ap()
    wu_d = nc.dram_tensor("wu", [128, NI * D], I8, kind="ExternalInput").ap()
    # wd packed d-major: [128, ND*I] int8, free slot d*I + i*128 + q  <-
    #   round(w[i*128+p, d*128+q]/s) for the [I, D] projection
    wd_d = nc.dram_tensor("wd", [128, NI * D], I8, kind="ExternalInput").ap()
    # cst[:, 0] = s_g (gate dequant scale, consumed only by Silu's scale)
    cst_d = nc.dram_tensor("cst", [128, 1], F32, kind="ExternalInput").ap()
    outT_d = nc.dram_tensor("outT", [D, C], BF, kind="ExternalOutput").ap()

    # PSUM bank holds 2KB/partition = 512 fp32: split the moving dim if needed.
    n_chunks = -(-C // 512)
    chunks = [(n * 512, min(512, C - n * 512)) for n in range(n_chunks)]

    with tile.TileContext(nc) as tc:
        with (
            tc.tile_pool(name="xp", bufs=1) as xp,
            tc.tile_pool(name="cp", bufs=1) as cp,
            tc.tile_pool(name="wp", bufs=1) as wp,
            tc.tile_pool(name="hp", bufs=1) as hp,
            tc.tile_pool(name="sp", bufs=3) as sp,
            tc.tile_pool(name="op", bufs=3) as op,
            tc.tile_pool(name="pg", bufs=3, space="PSUM") as pg,
            tc.tile_pool(name="pu", bufs=3, space="PSUM") as pu,
            tc.tile_pool(name="po", bufs=2, space="PSUM") as po,
        ):
            # Junk-burst weight tile: memset on vector (idle until the
            # first DVE mul) so gpsimd's first instruction is the wg0
            # doorbell.
            wj = cp.tile([128, 128], BF, tag="wj", name="w_junk")
            nc.vector.memset(wj[:], 0)

            # One x chunk per HWDGE ring: the first matmul gates on x01
            # + wg0 only.
            x01 = xp.tile([128, 2 * C], BF, tag="x01", name="x01")
            nc.sync.dma_start(x01[:], x01_d[:, :])
            x23 = xp.tile([128, 2 * C], BF, tag="x23", name="x23")
            nc.scalar.dma_start(x23[:], x23_d[:, :])
            x45 = xp.tile([128, 2 * C], BF, tag="x45", name="x45")
            nc.sync.dma_start(x45[:], x45_d[:, :])
            x67 = xp.tile([128, 2 * C], BF, tag="x67", name="x67")
            nc.scalar.dma_start(x67[:], x67_d[:, :])
            xt = [x01, x23, x45, x67]

            def xslice(d, c0, cn):
                return xt[d // 2][:, bass.ds((d % 2) * C + c0, cn)]

            # cst rides scalar behind x23: it's only needed by the first
            # Silu, which can lag the PE without stalling it.
            cst = cp.tile([128, 1], F32, tag="c", name="cst")
            nc.scalar.dma_start(cst[:], cst_d[:, :])

            # PE warm-up burst: junk matmuls from a zeroed tile into a
            # scratch rotation slot of the g PSUM pool, sized to overshoot
            # the first real matmul's data arrival (an idle gap would drop
            # the clock back to half rate).
            warm_ps = pg.tile([128, chunks[0][1]], F32, tag="g", name="warm_ps")
            for _ in range(N_JUNK):
                nc.tensor.matmul(
                    warm_ps[:, bass.ds(0, 64)],
                    wj[:],
                    wj[:, bass.ds(0, 64)],
                    start=True,
                    stop=True,
                )

            # SWDGE stream in consumption order, slack-maximized: the
            # i0 gate strip ships in two d-halves so the very first
            # matmuls gate on a 128KB transfer, and x45/x67 slot in
            # between the strips they feed.
            src = {"g": wg_d, "u": wu_d}
            smap = {}

            def ship_w(proj, b0, nb, d0=0, ndd=ND):
                t = wp.tile(
                    [128, nb * ndd * 128],
                    BF,
                    tag=f"w{proj}{b0}d{d0}",
                    name=f"w{proj}{b0}d{d0}",
                )
                nc.gpsimd.dma_start(
                    t[:], src[proj][:, bass.ds(b0 * D + d0 * 128, nb * ndd * 128)]
                )
                # contiguous [b0*D + d0*128, ...) covers slices i in
                # [b0, b0+nb) only when (d0, ndd) spans whole slices or
                # nb == 1; callers respect that.
                for i in range(b0, b0 + nb):
                    for d in range(d0, d0 + ndd):
                        smap[(proj, i, d)] = (
                            t,
                            (i - b0) * ndd * 128 + (d - d0) * 128,
                        )

            ship_w("g", 0, 1, 0, 4)
            ship_w("g", 0, 1, 4, 4)
            ship_w("u", 0, 1, 0, 4)
            ship_w("u", 0, 1, 4, 4)
            ship_w("g", 1, 1)
            ship_w("u", 1, 1)
            ship_w("g", 2, 2)
            ship_w("u", 2, 1)
            ship_w("u", 3, 1)
            for k in range(2, NI // 2):
                ship_w("g", 2 * k, 2)
                ship_w("u", 2 * k, 2)

            def wslice(proj, i, d):
                t, col = smap[(proj, i, d)]
                return t[:, bass.ds(col, 128)]

            # wd chunks ride at the stream tail in dd order.
            wd_t = []
            for dd in range(ND):
                t = wp.tile([128, I], BF, tag=f"wd{dd}", name=f"wd{dd}")
                nc.gpsimd.dma_start(t[:], wd_d[:, bass.ds(dd * I, I)])
                wd_t.append(t)

            def wdslice(i, dd):
                return wd_t[dd][:, bass.ds(i * 128, 128)]

            # Phase 1: hidT[i] = silu(Wg^T x^T) * (Wu^T x^T), one 128-row
            # strip of the intermediate dim per iteration. The matmuls see
            # dequant-scaled integers; ACT Silu folds s_g back in via its
            # pre-scale, so s_sb is the true silu(gate) and hT carries only
            # the 1/s_u factor (descaled on host together with s_d).
            hT = []
            for i in range(NI):
                h_t = hp.tile([128, C], BF, tag=f"h{i}", name=f"hT{i}")
                for c0, cn in chunks:
                    csl = bass.ds(c0, cn)
                    g_ps = pg.tile([128, cn], F32, tag="g", name="g_ps")
                    u_ps = pu.tile([128, cn], F32, tag="u", name="u_ps")
                    for d in range(ND):
                        nc.tensor.matmul(
                            g_ps[:],
                            wslice("g", i, d),
                            xslice(d, c0, cn),
                            start=(d == 0),
                            stop=(d == ND - 1),
                        )
                    for d in range(ND):
                        nc.tensor.matmul(
                            u_ps[:],
                            wslice("u", i, d),
                            xslice(d, c0, cn),
                            start=(d == 0),
                            stop=(d == ND - 1),
                        )
                    s_sb = sp.tile([128, cn], F32, tag="s", name="s_sb")
                    nc.scalar.activation(
                        s_sb[:],
                        g_ps[:],
                        mybir.ActivationFunctionType.Silu,
                        scale=cst[:, bass.ds(0, 1)],
                    )
                    nc.vector.tensor_mul(h_t[:, csl], s_sb[:], u_ps[:])
                hT.append(h_t)

            # Phase 2: outT[dstrip] = Wd^T @ hidT, accumulated over all 16
            # intermediate strips. Copies run on DVE (bf16 out); output
            # strips alternate between the two HWDGE rings (their cost is
            # per-row, so parallel queues halve the drain); the final
            # d-strip is split by partitions across both rings.
            for dd in range(ND):
                last = dd == ND - 1
                for c0, cn in chunks:
                    # the final d-strip runs in two column chunks so its
                    # first half's copy+DMA overlaps the second half's
                    # accumulation
                    if last:
                        ch = cn // 2
                        sub = [(c0, ch), (c0 + ch, cn - ch)]
                    else:
                        sub = [(c0, cn)]
                    for s0, sn in sub:
                        ssl = bass.ds(s0, sn)
                        o_ps = po.tile([128, sn], F32, tag="o", name="o_ps")
                        for i in range(NI):
                            nc.tensor.matmul(
                                o_ps[:],
                                wdslice(i, dd),
                                hT[i][:, ssl],
                                start=(i == 0),
                                stop=(i == NI - 1),
                            )
                        o_sb = op.tile([128, sn], BF, tag="ob", name="o_sb")
                        nc.vector.tensor_scalar_mul(o_sb[:], o_ps[:], 1.0)
                        h0 = bass.ds(dd * 128, 64)
                        h1 = bass.ds(dd * 128 + 64, 64)
                        nc.scalar.dma_start(outT_d[h0, ssl], o_sb[0:64, :])
                        nc.sync.dma_start(outT_d[h1, ssl], o_sb[64:128, :])

    nc.compile()
    return nc


def _get_program(C):
    if C not in _PROGRAM_CACHE:
        _PROGRAM_CACHE[C] = _build_program(C)
    return _PROGRAM_CACHE[C]


def _run(nc, in_maps, trace=False):
    from concourse.bass_utils import run_bass_kernel_spmd

    return run_bass_kernel_spmd(nc, in_maps, core_ids=list(range(E)), trace=trace)


def _quant8(w):
    # per-expert-tensor linear int8 quantization; returns (q, scale)
    s = float(np.abs(w).max()) / 127.0
    if s == 0.0:
        s = 1.0
    q = np.clip(np.rint(w / s), -127, 127).astype(np.int8)
    return q, s


def _pack_w(q, transpose):
    # transpose=True (wg/wu, [D, I]): -> [128, NI*D], free slot i*D + d*128 + q,
    #   block (i,d) = w[d*128:+128, i*128:+128]
    # transpose=False (wd, [I, D]): -> [128, ND*I] d-major, free slot
    #   d*I + i*128 + q, block (i,d) = w[i*128:+128, d*128:+128]
    if transpose:
        b = q.reshape(ND, 128, NI, 128).transpose(1, 2, 0, 3)  # p, i, d, q
    else:
        b = q.reshape(NI, 128, ND, 128).transpose(1, 2, 0, 3)  # p, d, i, q
    return np.ascontiguousarray(b.reshape(128, NI * D))


def _kernel_numpy(x, idx, w_gate, w_up, w_down):
    # exact fallback for pathological token skew (SBUF can't hold >~1536
    # tokens per expert); normal inputs never take this path
    out = np.zeros((T, D), dtype=np.float32)
    for e in range(E):
        m = idx == e
        if not m.any():
            continue
        g = x[m] @ w_gate[e]
        u = x[m] @ w_up[e]
        out[m] = (g / (1.0 + np.exp(-g)) * u) @ w_down[e]
    return out


def kernel(x, expert_indices, w_gate, w_up, w_down, _trace=False, _results=None):
    x = np.asarray(x)
    idx = np.asarray(expert_indices).astype(np.int64)
    counts = np.bincount(idx, minlength=E)
    C = int(max(128, -(-counts.max() // 2) * 2))
    if C > 1536:
        return _kernel_numpy(
            x, idx, np.asarray(w_gate), np.asarray(w_up), np.asarray(w_down)
        )

    nc = _get_program(C)

    order = np.argsort(idx, kind="stable")
    starts = np.zeros(E + 1, dtype=np.int64)
    np.cumsum(counts, out=starts[1:])

    bf16 = ml_dtypes.bfloat16
    in_maps = []
    scales = []
    for e in range(E):
        toks = order[starts[e] : starts[e + 1]]
        # xT packed: [p, d, c] = x[tok c, d*128+p]
        xTg = np.zeros((128, ND, C), dtype=bf16)
        xTg[:, :, : len(toks)] = (
            x[toks].astype(bf16).T.reshape(ND, 128, len(toks)).transpose(1, 0, 2)
        )
        qg, sg = _quant8(np.asarray(w_gate[e]))
        qu, su = _quant8(np.asarray(w_up[e]))
        qd, sd = _quant8(np.asarray(w_down[e]))
        scales.append(su * sd)
        cst = np.full((128, 1), sg, dtype=np.float32)
        in_maps.append(
            {
                "x01": np.ascontiguousarray(xTg[:, 0:2, :].reshape(128, 2 * C)),
                "x23": np.ascontiguousarray(xTg[:, 2:4, :].reshape(128, 2 * C)),
                "x45": np.ascontiguousarray(xTg[:, 4:6, :].reshape(128, 2 * C)),
                "x67": np.ascontiguousarray(xTg[:, 6:8, :].reshape(128, 2 * C)),
                "wg": _pack_w(qg, True),
                "wu": _pack_w(qu, True),
                "wd": _pack_w(qd, False),
                "cst": cst,
            }
        )

    res = _run(nc, in_maps, trace=_trace)
    if _results is not None:
        _results.append(res)

    out = np.zeros((T, D), dtype=np.float32)
    for e in range(E):
        toks = order[starts[e] : starts[e + 1]]
        outT = res.results[e]["outT"]  # [D, C] bf16, unscaled by s_u*s_d
        out[toks] = outT[:, : len(toks)].T.astype(np.float32) * scales[e]
    return out


# revision 25
# speedup vs baseline: 1.0169x; 1.0169x over previous
"""MoE expert-group kernel for Trainium2 (8 NeuronCores).

Problem: T=2048 tokens, E=8 experts, D=1024, I=2048.
  out[t] = silu(x[t] @ w_gate[e]) * (x[t] @ w_up[e]) @ w_down[e],  e = expert_indices[t]

Strategy: expert parallelism. Host-side (numpy) routing gathers tokens by
expert (this is the "all-to-all"); core e runs expert e's dense
gate/up/silu/down pipeline; host scatters rows back.

On-chip formulation is fully transposed so no transposes are ever needed:
  gateT = Wg^T @ X^T        (stationary = 128x128 Wg block, moving = xT [128, C])
  hidT  = silu(gateT)*upT   (ACT Silu + one DVE mul, written bf16)
  outT  = Wd^T @ hidT       (stationary = 128x128 Wd block, moving = hT [128, C])

Numerics: weights are shipped as int8 (per-expert linear quantization,
scale = absmax/127) and cast to bf16 *inside* the gpsimd DMA engines, so
HBM weight traffic halves while the PE still runs plain bf16 matmuls on
exact integer values. x stays bf16. The gate dequant scale s_g is folded
into the Silu's ACT pre-scale (it sits inside the nonlinearity); the
remaining 1/(s_u*s_d) factor is purely multiplicative, so the output is
written back bf16 UNscaled and the host applies s_u*s_d during the
fp32 upcast. Measured rel err ~8e-3 vs the 2e-2 budget.

Timeline engineering (what the traces showed matters on TRN2):
- The PE clock sits at 1.2GHz until ~3.2us of CONTINUOUS busy, then
  jumps to full rate (HAM 8/8); ANY idle gap drops it back and costs
  ~3.4us of half-rate work. A junk-matmul burst (zeros from a memset
  tile into a scratch PSUM slot) starts right at body entry and is
  sized to overshoot the first real matmul's data arrival, so the
  handoff is seamless and all real matmuls run at full clock.
- DMA queues have ~17ns/row descriptor overhead, so x tiles ([128, C]
  with 0.5-1KB rows) are row-bound, not byte-bound: one x chunk per
  HWDGE ring (sync: d0d1, scalar: d2d3), and d4..d7 ride the SWDGE
  casting ring (fast per-packet) right after the first gate strip.
  Each dma_start also costs ~0.7us of ISSUE time on its engine queue,
  so the doorbell order is part of the schedule.
- SWDGE stream order == consumption order with slack maximized:
  wg0, x4567, wu0, wg1, wu1, then 2-slice pairs, then wd d-major.
- The measured window ends only after a fixed ~8us walrus postamble
  (all-engine barrier + full semaphore-file reset + barrier), so the
  controllable tail is the output drain: copies run bf16 on DVE,
  output strips alternate between the two HWDGE rings, and the final
  d-strip is split by PARTITIONS across both rings (column splits
  don't help: DMA cost is per-row).
"""

import sys

import numpy as np

try:
    import concourse  # noqa: F401
except ImportError:  # grading env fallback
    sys.path.insert(0, "/opt/trn_rl_repo")

import ml_dtypes

T, E, D, I = 2048, 8, 1024, 2048
ND = D // 128  # 8 contraction tiles for gate/up
NI = I // 128  # 16 contraction tiles for down
N_JUNK = 78  # PE warm-up burst length (64-col junk matmuls, ~53ns each)

_PROGRAM_CACHE = {}


def _build_program(C):
    """Build + compile the per-core Bass program for token capacity C."""
    import concourse.bass as bass  # noqa: F401
    import concourse.mybir as mybir
    import concourse.tile as tile
    from concourse import bacc

    BF = mybir.dt.bfloat16
    I8 = mybir.dt.int8
    F32 = mybir.dt.float32

    nc = bacc.Bacc(
        "TRN2",
        target_bir_lowering=False,
        debug=False,
        num_devices=E,
        enable_partition_id=False,
    )
    # x ships as 3 tensors: x01/x23 = [128, 2*C] (d-pairs, slot d'*C+c ->
    # x[tok c, (2j+d')*128+p]) on the two HWDGE rings (they finish before
    # the SWDGE head needs the HBM port), x4567 = [128, 4*C] inside the
    # SWDGE stream (the port is a shared ~358GB/s budget: parallel queues
    # just steal from the weight stream, serialization wins)
    x01_d = nc.dram_tensor("x01", [128, 2 * C], BF, kind="ExternalInput").ap()
    x23_d = nc.dram_tensor("x23", [128, 2 * C], BF, kind="ExternalInput").ap()
    x4567_d = nc.dram_tensor("x4567", [128, 4 * C], BF, kind="ExternalInput").ap()
    # wg/wu packed: [128, NI*D] int8, free slot i*D + d*128 + q  <-
    #   round(w[d*128+p, i*128+q]/s) for the [D, I] projections
    wg_d = nc.dram_tensor("wg", [128, NI * D], I8, kind="ExternalInput").ap()
    wu_d = nc.dram_tensor("wu", [128, NI * D], I8, kind="ExternalInput").ap()
    # wd packed d-major: [128, ND*I] int8, free slot d*I + i*128 + q  <-
    #   round(w[i*128+p, d*128+q]/s) for the [I, D] projection
    wd_d = nc.dram_tensor("wd", [128, NI * D], I8, kind="ExternalInput").ap()
    # cst[:, 0] = s_g (gate dequant scale, consumed only by Silu's scale)
    cst_d = nc.dram_tensor("cst", [128, 1], F32, kind="ExternalInput").ap()
    outT_d = nc.dram_tensor("outT", [D, C], BF, kind="ExternalOutput").ap()

    # PSUM bank holds 2KB/partition = 512 fp32: split the moving dim if needed.
    n_chunks = -(-C // 512)
    chunks = [(n * 512, min(512, C - n * 512)) for n in range(n_chunks)]

    with tile.TileContext(nc) as tc:
        with (
            tc.tile_pool(name="xp", bufs=1) as xp,
            tc.tile_pool(name="cp", bufs=1) as cp,
            tc.tile_pool(name="wp", bufs=1) as wp,
            tc.tile_pool(name="hp", bufs=1) as hp,
            tc.tile_pool(name="sp", bufs=3) as sp,
            tc.tile_pool(name="op", bufs=3) as op,
            tc.tile_pool(name="pg", bufs=3, space="PSUM") as pg,
            tc.tile_pool(name="pu", bufs=3, space="PSUM") as pu,
            tc.tile_pool(name="po", bufs=2, space="PSUM") as po,
        ):
            # Junk-burst weight tile: zeroed via ACT (scalar enters the
            # body early and its queue is otherwise free) so the PE busy
            # window starts ASAP; gpsimd's first instruction stays the
            # wg0 doorbell.
            wj = cp.tile([128, 128], BF, tag="wj", name="w_junk")
            nc.scalar.memzero(wj[:])

            # One x chunk per HWDGE ring: the first matmul gates on x01
            # + wg0 only.
            x01 = xp.tile([128, 2 * C], BF, tag="x01", name="x01")
            nc.sync.dma_start(x01[:], x01_d[:, :])
            x23 = xp.tile([128, 2 * C], BF, tag="x23", name="x23")
            nc.scalar.dma_start(x23[:], x23_d[:, :])
            x4567 = xp.tile([128, 4 * C], BF, tag="x4567", name="x4567")
            xt = [x01, x23, x4567]

            def xslice(d, c0, cn):
                if d < 4:
                    return xt[d // 2][:, bass.ds((d % 2) * C + c0, cn)]
                return xt[2][:, bass.ds((d - 4) * C + c0, cn)]

            # cst rides scalar behind x23: it's only needed by the first
            # Silu, which can lag the PE without stalling it.
            cst = cp.tile([128, 1], F32, tag="c", name="cst")
            nc.scalar.dma_start(cst[:], cst_d[:, :])

            # PE warm-up burst: junk matmuls from a zeroed tile into a
            # scratch rotation slot of the g PSUM pool, sized to overshoot
            # the first real matmul's data arrival (an idle gap would drop
            # the clock back to half rate).
            warm_ps = pg.tile([128, chunks[0][1]], F32, tag="g", name="warm_ps")
            for _ in range(N_JUNK):
                nc.tensor.matmul(
                    warm_ps[:, bass.ds(0, 64)],
                    wj[:],
                    wj[:, bass.ds(0, 64)],
                    start=True,
                    stop=True,
                )

            # SWDGE stream in consumption order, slack-maximized: the
            # i0 gate strip ships in two d-halves so the very first
            # matmuls gate on a 128KB transfer, and x45/x67 slot in
            # between the strips they feed.
            src = {"g": wg_d, "u": wu_d}
            smap = {}

            def ship_w(proj, b0, nb, d0=0, ndd=ND):
                t = wp.tile(
                    [128, nb * ndd * 128],
                    BF,
                    tag=f"w{proj}{b0}d{d0}",
                    name=f"w{proj}{b0}d{d0}",
                )
                nc.gpsimd.dma_start(
                    t[:], src[proj][:, bass.ds(b0 * D + d0 * 128, nb * ndd * 128)]
                )
                # contiguous [b0*D + d0*128, ...) covers slices i in
                # [b0, b0+nb) only when (d0, ndd) spans whole slices or
                # nb == 1; callers respect that.
                for i in range(b0, b0 + nb):
                    for d in range(d0, d0 + ndd):
                        smap[(proj, i, d)] = (
                            t,
                            (i - b0) * ndd * 128 + (d - d0) * 128,
                        )

            ship_w("g", 0, 1)
            nc.gpsimd.dma_start(x4567[:], x4567_d[:, :])
            ship_w("u", 0, 1)
            ship_w("g", 1, 1)
            ship_w("u", 1, 1)
            ship_w("g", 2, 2)
            ship_w("u", 2, 1)
            ship_w("u", 3, 1)
            for k in range(2, NI // 2):
                ship_w("g", 2 * k, 2)
                ship_w("u", 2 * k, 2)

            def wslice(proj, i, d):
                t, col = smap[(proj, i, d)]
                return t[:, bass.ds(col, 128)]

            # wd chunks ride at the stream tail in dd order.
            wd_t = []
            for dd in range(ND):
                t = wp.tile([128, I], BF, tag=f"wd{dd}", name=f"wd{dd}")
                nc.gpsimd.dma_start(t[:], wd_d[:, bass.ds(dd * I, I)])
                wd_t.append(t)

            def wdslice(i, dd):
                return wd_t[dd][:, bass.ds(i * 128, 128)]

            # Phase 1: hidT[i] = silu(Wg^T x^T) * (Wu^T x^T), one 128-row
            # strip of the intermediate dim per iteration. The matmuls see
            # dequant-scaled integers; ACT Silu folds s_g back in via its
            # pre-scale, so s_sb is the true silu(gate) and hT carries only
            # the 1/s_u factor (descaled on host together with s_d).
            hT = []
            for i in range(NI):
                h_t = hp.tile([128, C], BF, tag=f"h{i}", name=f"hT{i}")
                for c0, cn in chunks:
                    csl = bass.ds(c0, cn)
                    g_ps = pg.tile([128, cn], F32, tag="g", name="g_ps")
                    u_ps = pu.tile([128, cn], F32, tag="u", name="u_ps")
                    for d in range(ND):
                        nc.tensor.matmul(
                            g_ps[:],
                            wslice("g", i, d),
                            xslice(d, c0, cn),
                            start=(d == 0),
                            stop=(d == ND - 1),
                        )
                    for d in range(ND):
                        nc.tensor.matmul(
                            u_ps[:],
                            wslice("u", i, d),
                            xslice(d, c0, cn),
                            start=(d == 0),
                            stop=(d == ND - 1),
                        )
                    s_sb = sp.tile([128, cn], F32, tag="s", name="s_sb")
                    nc.scalar.activation(
                        s_sb[:],
                        g_ps[:],
                        mybir.ActivationFunctionType.Silu,
                        scale=cst[:, bass.ds(0, 1)],
                    )
                    nc.vector.tensor_mul(h_t[:, csl], s_sb[:], u_ps[:])
                hT.append(h_t)

            # Phase 2: outT[dstrip] = Wd^T @ hidT, accumulated over all 16
            # intermediate strips. Copies run on DVE (bf16 out); output
            # strips alternate between the two HWDGE rings (their cost is
            # per-row, so parallel queues halve the drain); the final
            # d-strip is split by partitions across both rings.
            for dd in range(ND):
                last = dd == ND - 1
                for c0, cn in chunks:
                    # the final d-strip runs in two column chunks so its
                    # first half's copy+DMA overlaps the second half's
                    # accumulation
                    if last:
                        ch = cn // 2
                        sub = [(c0, ch), (c0 + ch, cn - ch)]
                    else:
                        sub = [(c0, cn)]
                    for s0, sn in sub:
                        ssl = bass.ds(s0, sn)
                        o_ps = po.tile([128, sn], F32, tag="o", name="o_ps")
                        for i in range(NI):
                            nc.tensor.matmul(
                                o_ps[:],
                                wdslice(i, dd),
                                hT[i][:, ssl],
                                start=(i == 0),
                                stop=(i == NI - 1),
                            )
                        o_sb = op.tile([128, sn], BF, tag="ob", name="o_sb")
                        nc.vector.tensor_scalar_mul(o_sb[:], o_ps[:], 1.0)
                        h0 = bass.ds(dd * 128, 64)
                        h1 = bass.ds(dd * 128 + 64, 64)
                        nc.scalar.dma_start(outT_d[h0, ssl], o_sb[0:64, :])
                        nc.sync.dma_start(outT_d[h1, ssl], o_sb[64:128, :])

    nc.compile()
    return nc


def _get_program(C):
    if C not in _PROGRAM_CACHE:
        _PROGRAM_CACHE[C] = _build_program(C)
    return _PROGRAM_CACHE[C]


def _run(nc, in_maps, trace=False):
    from concourse.bass_utils import run_bass_kernel_spmd

    return run_bass_kernel_spmd(nc, in_maps, core_ids=list(range(E)), trace=trace)


def _quant8(w):
    # per-expert-tensor linear int8 quantization; returns (q, scale)
    s = float(np.abs(w).max()) / 127.0
    if s == 0.0:
        s = 1.0
    q = np.clip(np.rint(w / s), -127, 127).astype(np.int8)
    return q, s


def _pack_w(q, transpose):
    # transpose=True (wg/wu, [D, I]): -> [128, NI*D], free slot i*D + d*128 + q,
    #   block (i,d) = w[d*128:+128, i*128:+128]
    # transpose=False (wd, [I, D]): -> [128, ND*I] d-major, free slot
    #   d*I + i*128 + q, block (i,d) = w[i*128:+128, d*128:+128]
    if transpose:
        b = q.reshape(ND, 128, NI, 128).transpose(1, 2, 0, 3)  # p, i, d, q
    else:
        b = q.reshape(NI, 128, ND, 128).transpose(1, 2, 0, 3)  # p, d, i, q
    return np.ascontiguousarray(b.reshape(128, NI * D))


def _kernel_numpy(x, idx, w_gate, w_up, w_down):
    # exact fallback for pathological token skew (SBUF can't hold >~1536
    # tokens per expert); normal inputs never take this path
    out = np.zeros((T, D), dtype=np.float32)
    for e in range(E):
        m = idx == e
        if not m.any():
            continue
        g = x[m] @ w_gate[e]
        u = x[m] @ w_up[e]
        out[m] = (g / (1.0 + np.exp(-g)) * u) @ w_down[e]
    return out


def kernel(x, expert_indices, w_gate, w_up, w_down, _trace=False, _results=None):
    x = np.asarray(x)
    idx = np.asarray(expert_indices).astype(np.int64)
    counts = np.bincount(idx, minlength=E)
    C = int(max(128, -(-counts.max() // 2) * 2))
    if C > 1536:
        return _kernel_numpy(
            x, idx, np.asarray(w_gate), np.asarray(w_up), np.asarray(w_down)
        )

    nc = _get_program(C)

    order = np.argsort(idx, kind="stable")
    starts = np.zeros(E + 1, dtype=np.int64)
    np.cumsum(counts, out=starts[1:])

    bf16 = ml_dtypes.bfloat16
    in_maps = []
    scales = []
    for e in range(E):
        toks = order[starts[e] : starts[e + 1]]
        # xT packed: [p, d, c] = x[tok c, d*128+p]
        xTg = np.zeros((128, ND, C), dtype=bf16)
        xTg[:, :, : len(toks)] = (
            x[toks].astype(bf16).T.reshape(ND, 128, len(toks)).transpose(1, 0, 2)
        )
        qg, sg = _quant8(np.asarray(w_gate[e]))
        qu, su = _quant8(np.asarray(w_up[e]))
        qd, sd = _quant8(np.asarray(w_down[e]))
        scales.append(su * sd)
        cst = np.full((128, 1), sg, dtype=np.float32)
        in_maps.append(
            {
                "x01": np.ascontiguousarray(xTg[:, 0:2, :].reshape(128, 2 * C)),
                "x23": np.ascontiguousarray(xTg[:, 2:4, :].reshape(128, 2 * C)),
                "x4567": np.ascontiguousarray(xTg[:, 4:8, :].reshape(128, 4 * C)),
                "wg": _pack_w(qg, True),
                "wu": _pack_w(qu, True),
                "wd": _pack_w(qd, False),
                "cst": cst,
            }
        )

    res = _run(nc, in_maps, trace=_trace)
    if _results is not None:
        _results.append(res)

    out = np.zeros((T, D), dtype=np.float32)
    for e in range(E):
        toks = order[starts[e] : starts[e + 1]]
        outT = res.results[e]["outT"]  # [D, C] bf16, unscaled by s_u*s_d
        out[toks] = outT[:, : len(toks)].T.astype(np.float32) * scales[e]
    return out
